# revision 47
# baseline (speedup 1.0000x reference)
"""Trainium2 Bass kernel for GQA multi-head attention (B=2, S=2048, D=2048,
16 Q heads / 4 KV heads, head_dim=128, RoPE, causal). bf16 compute, fp32 accum.

Sharding: 8 cores = 2 (batch) x 4 (tensor-parallel head groups).
Each core: 4 Q heads + 1 KV head for one batch element; partial output
projection [D, S]; host sums the 4 TP partials per batch element.

Per-core dataflow (bf16 in SBUF, fp32 in PSUM; PSUM = one pool, two tags:
sc 2x[128,1024], av 4x[128,512] — rings shared across phases so nothing
ever waits on a pool boundary):

  opening: DMAs are consolidated into few large transfers (HWDGE costs
    ~630ns per dma_start, serialized) issued in consumption order on the
    in-order sync queue; sq0's x/wq stream in 4-chunk groups. ~10 junk
    matmuls on a zeroed tile warm the PE's HAM clock gate while the
    opening DMAs land.
  phase 1 (per s-quarter sq): sq0 runs chunk-major (all 4 Q products per
    dc-chunk as it arrives, accumulators in the sc pairs); sq1-3 run six
    sequential single-bank passes (Q0..Q3, K, then V). V is computed
    directly in natural [s, dv] blocks (stationary = xt block, moving =
    wv chunk; one psum alloc per block — group tracking is bank-granular)
    so no PE transposes are needed. Q/K drains alternate engines
    (ACT drain + Pool rope, or DVE drain + DVE rope):
      qs/qsw copies then dst = qs*cos + swap(qs)*sin_negated.
    K's drain is emitted after the V pass so VnO copies hit an idle ACT.
    During sq3, the qg0 sections' score pairs are sprinkled between
    passes so their exps retire early and phase 2 opens with AV work.
  phase 2 (per section (qg, h), software-pipelined slots): slot t emits
    AV blocks of sec_{t-1} (order 0,2,1,3 so consecutive blocks hit
    different psum banks), out-projection series, and scores(sec_t),
    keeping the PE busy while ACT works through the exp backlog.
    scores: per 128-k-chunk-PAIR [128, 1024] psum, diagonal chunks first;
    one wide exp covers both valid regions (per-instr ACT overhead is
    ~185ns, pairing halves it); diagonal blocks masked on Pool.
    AV flipped: pav[q, 129] += at_block.T @ VnO_chunk[dv | ones]
      -> softmax denominator rides in column 128 for free.
    normalize: aot = pav[:,0:128] * reciprocal(pav[:,128]) (DVE);
    deferred PE-transpose aot -> AO[h] [dv, q] one slot later (targets are
    spare columns of the pav banks, bitcast bf16).
  phase 3: outT[Dc, 512q] = sum_hc wo_chunk.T @ AO[hc], emitted as soon as
    each q-quarter's AO completes, interleaved between phase-2 AV blocks;
    drains on DVE; consecutive Dc pairs share one output DMA.

Output: per-core outT [D, S] bf16; host sums out[b] = sum_tp outT.T
"""

import numpy as np
from contextlib import ExitStack

import ml_dtypes
import concourse.bass as bass
import concourse.mybir as mybir
from concourse import bacc, tile
from concourse.bass_utils import run_bass_kernel_spmd
from concourse.masks import make_identity, make_upper_triangular

F32 = mybir.dt.float32
BF16 = mybir.dt.bfloat16
AF = mybir.ActivationFunctionType

S = 2048
D = 2048
P = 128
NHQ = 4   # q heads per core
N_CORES = 8
N_TP = 4
VW = 132  # VnO column stride per k-chunk: 128 dv + 1 ones + 3 pad


def _build_kernel(nc, tc, ctx, xT, wq, wkv, wo, cos2, sin2n, outT):
    const = ctx.enter_context(tc.tile_pool(name="const", bufs=1))
    xtp = ctx.enter_context(tc.tile_pool(name="xtp", bufs=2))
    sbp = ctx.enter_context(tc.tile_pool(name="sbp", bufs=8))
    ropep = ctx.enter_context(tc.tile_pool(name="ropep", bufs=6))
    atp = ctx.enter_context(tc.tile_pool(name="atp", bufs=16))
    aotp = ctx.enter_context(tc.tile_pool(name="aotp", bufs=9))
    recp = ctx.enter_context(tc.tile_pool(name="recp", bufs=9))
    obp = ctx.enter_context(tc.tile_pool(name="obp", bufs=6))

    # ---- persistent weights / tables ----
    wqc = const.tile([P, 16 * 512], BF16, tag="wqc")
    wkvc = const.tile([P, 16 * 256], BF16, tag="wkvc")
    cos2t = const.tile([P, S], BF16, tag="cos2t")
    sin2nt = const.tile([P, S], BF16, tag="sin2nt")
    woc = const.tile([P, 4 * D], BF16, tag="woc")

    # DMA kickoff in consumption order, consolidated into few transfers (the
    # HWDGE descriptor engine costs ~630ns per dma_start, serialized): sq0's
    # x and wq stream in 4-chunk groups so pass A starts early; everything
    # else is one transfer. HBM-side chunking via einops rearrange.
    wup0 = const.tile([P, 512], BF16, tag="wup")
    nc.gpsimd.memset(wup0[:], 0.0)

    wqr = wq.rearrange("(dc p) c -> p dc c", p=P)      # [128, 16, 512]
    wkvr = wkv.rearrange("(dc p) c -> p dc c", p=P)    # [128, 16, 256]
    wor = wo.rearrange("(hc p) c -> p hc c", p=P)      # [128, 4, 2048]
    xtr = xT.rearrange("(dc p) s -> p dc s", p=P)      # [128, 16, 2048]
    xtb = {0: xtp.tile([P, 16 * 512], BF16, tag="xtb", name="xtb0")}
    for g in range(4):
        nc.sync.dma_start(
            xtb[0][:, 2048 * g : 2048 * (g + 1)],
            xtr[:, 4 * g : 4 * (g + 1), 0:512],
        )
        nc.sync.dma_start(
            wqc[:, 2048 * g : 2048 * (g + 1)], wqr[:, 4 * g : 4 * (g + 1), :]
        )
    nc.sync.dma_start(wkvc[:], wkvr[:])
    nc.sync.dma_start(cos2t[:], cos2[:])
    nc.sync.dma_start(sin2nt[:], sin2n[:])

    # ---- PSUM: one pool, two tags; 8 banks total ----
    # sc: 2x[128,1024] (sq0 Q pairs, then score chunk-pairs)
    # av: 4x[128,512] (phase-1 accumulators/V blocks, then AV + out-proj)
    ps = ctx.enter_context(tc.tile_pool(name="ps", bufs=1, space="PSUM"))

    # PE warm-up: the tensor engine's HAM clock gate holds the PE at half
    # clock until ~3.4us of sustained activity. The opening is DMA-paced, so
    # burn that window on junk matmuls (zeros) instead of on the real ones.
    wup = wup0
    pw = ps.tile([P, 512], F32, tag="av", bufs=4, name="pw")
    for i in range(10):12        nc.tensor.matmul(12            pw[:], wup[:, 0:128], wup[:], start=True, stop=True,
            skip_group_check=True,
        )

    maskt = const.tile([P, P], BF16, tag="maskt")
    make_upper_triangular(nc, maskt[:], val=1.0, diag=True)
    ident = const.tile([P, P], BF16, tag="ident")
    make_identity(nc, ident[:])

    QT = [const.tile([P, S], BF16, tag=f"QT{i}", name=f"QT{i}") for i in range(NHQ)]
    KT = const.tile([P, S], BF16, tag="KT")
    # VnO: per k-chunk c: cols [VW*c, VW*c+128) = V rows [128c,+128) x dv,
    # col VW*c+128 = ones (softmax denominator rides the AV matmul)
    VnO = const.tile([P, 16 * VW], BF16, tag="VnO")
    for c in range(16):
        nc.gpsimd.memset(VnO[:, VW * c + 128 : VW * c + 129], 1.0)
    AO = [const.tile([P, S], BF16, tag=f"AO{i}", name=f"AO{i}") for i in range(NHQ)]

    pre_score_thunks, run_phase23 = _make_phase23(
        nc, ps, atp, aotp, recp, obp, maskt, ident, QT, KT, VnO, AO, woc, outT
    )
    _phase1(nc, ps, xtp, sbp, ropep, xtr, xtb, wqc, wkvc, cos2t, sin2nt,
            QT, KT, VnO, wor, woc, pre_score_thunks)
    run_phase23()


def _phase1(nc, ps, xtp, sbp, ropep, xtr, xtb, wqc, wkvc, cos2t, sin2nt,
            QT, KT, VnO, wor, woc, pre_score_thunks):
    def drain_rope(psrc, dst, sl, alt):
        # psum -> bf16 sbuf: natural copy + half-swapped copy, then RoPE as
        # 3 full-width all-bf16 ops: dst = qs*cos + swap(qs)*sin_negated.
        # Engines alternate per tensor (drain ACT + rope Pool, or both DVE)
        # so no single engine's queue backlog stalls the psum bank recycle.
        qs = sbp.tile([P, 512], BF16, tag="qs")
        qsw = sbp.tile([P, 512], BF16, tag="qsw")
        if alt % 2 == 0:
            nc.scalar.activation(qs[:], psrc, AF.Copy)
            nc.scalar.activation(qsw[0:64, :], psrc[64:128, :], AF.Copy)
            nc.scalar.activation(qsw[64:128, :], psrc[0:64, :], AF.Copy)
            reng = nc.gpsimd
        else:
            nc.vector.tensor_scalar_add(qs[:], psrc, 0.0)
            nc.vector.tensor_scalar_add(qsw[0:64, :], psrc[64:128, :], 0.0)
            nc.vector.tensor_scalar_add(qsw[64:128, :], psrc[0:64, :], 0.0)
            reng = nc.vector
        m1 = ropep.tile([P, 512], BF16, tag="m1")
        m2 = ropep.tile([P, 512], BF16, tag="m2")
        reng.tensor_mul(m1[:], qs[:], cos2t[:, sl])
        reng.tensor_mul(m2[:], qsw[:], sin2nt[:, sl])
        reng.tensor_add(dst[:, sl], m1[:], m2[:])

    for sq in range(4):
        if sq == 3:
            # qg0 sections need only sq0 data: their score pairs get
            # sprinkled between sq3's passes below, so the exps retire on
            # ACT during sq3 and phase 2 opens with ready AV work
            pre = []
            for h0 in range(NHQ):
                pre.extend(pre_score_thunks(0, h0))
        else:
            pre = []
        s0 = 512 * sq
        sl = slice(s0, s0 + 512)
        # prefetch next sq's x square in one transfer (xtb double-buffers)
        if sq < 3:
            xtb[sq + 1] = xtp.tile([P, 16 * 512], BF16, tag="xtb",
                                   name=f"xtb{sq+1}")
            nc.sync.dma_start(
                xtb[sq + 1][:], xtr[:, :, 512 * (sq + 1) : 512 * (sq + 2)]
            )
        if sq == 1:
            # stream the phase-3 weights while the DMA queues are quiet
            # (on sync: its in-order queue keeps this behind the opening)
            nc.sync.dma_start(woc[:], wor[:])
        xtq = xtb.pop(sq)
        xtl = [xtq[:, 512 * dc : 512 * (dc + 1)] for dc in range(16)]

        def pass_V():
            # pass V: natural [s, dv] blocks; stationary = xt block, moving
            # = wv. Each t-block gets its own psum alloc: accumulation-group
            # tracking is bank-granular, so sharing one tile would serialize
            # block t+1's matmuls behind block t's VnO drain.
            for t in range(4):
                pVt = ps.tile([P, 512], F32, tag="av", bufs=4, name="pVt")
                for dc in range(16):
                    nc.tensor.matmul(
                        pVt[:, 0:128],
                        xtq[:, 512 * dc + 128 * t : 512 * dc + 128 * (t + 1)],
                        wkvc[:, 256 * dc + 128 : 256 * dc + 256],
                        start=dc == 0, stop=dc == 15, skip_group_check=True,
                    )
                j = 4 * sq + t
                if t % 2 == 0:
                    nc.scalar.activation(
                        VnO[:, VW * j : VW * j + 128], pVt[:, 0:128], AF.Copy,
                    )
                else:
                    nc.vector.tensor_scalar_add(
                        VnO[:, VW * j : VW * j + 128], pVt[:, 0:128], 0.0,
                    )

        if sq == 0:
            # sq0 is paced by the opening DMA stream: consume chunks in
            # chunk-major order (all 4 Q products per chunk as it arrives).
            # Borrows the phase-2 'sc' tag for the 4 live accumulators —
            # phase 2 is far away and the drains finish during sq1.
            pQp = [
                ps.tile([P, 1024], F32, tag="sc", bufs=2, name=f"pQp{i}")
                for i in range(2)
            ]
            qt = [pQp[h // 2][:, 512 * (h % 2) : 512 * (h % 2) + 512]
                  for h in range(NHQ)]
            for dc in range(16):
                for h in range(NHQ):
                    nc.tensor.matmul(
                        qt[h],
                        wqc[:, 512 * dc + 128 * h : 512 * dc + 128 * (h + 1)],
                        xtl[dc], start=dc == 0, stop=dc == 15,
                        skip_group_check=True,
                    )
            for h in range(NHQ):
                drain_rope(qt[h], QT[h], sl, h)
        else:
            # passes Q0..Q3: single-bank accumulation, drain+rope per pass
            for h in range(NHQ):
                pQ = ps.tile([P, 512], F32, tag="av", bufs=4, name=f"pQ{h}")
                for dc in range(16):
                    nc.tensor.matmul(
                        pQ[:],
                        wqc[:, 512 * dc + 128 * h : 512 * dc + 128 * (h + 1)],
                        xtl[dc], start=dc == 0, stop=dc == 15,
                    )
                drain_rope(pQ[:], QT[h], sl, h)
                for _ in range(2):
                    if pre:
                        pre.pop(0)()
        pK = ps.tile([P, 512], F32, tag="av", bufs=4, name="pK")
        for dc in range(16):
            nc.tensor.matmul(
                pK[:], wkvc[:, 256 * dc : 256 * dc + 128], xtl[dc],
                start=dc == 0, stop=dc == 15,
            )
        for _ in range(2):
            if pre:
                pre.pop(0)()
        pass_V()
        # K's drain is emitted after the V pass so the VnO copies reach an
        # empty ACT queue (KT for this sq isn't needed until phase 2)
        drain_rope(pK[:], KT, sl, 0)
        for t in pre:
            t()


def _make_phase23(nc, ps, atp, aotp, recp, obp, maskt, ident, QT, KT, VnO,
                  AO, woc, outT):
    secs = [(qg, h) for qg in range(4) for h in range(4)]
    po_queue = []    # (Dc, qtr) output-projection series ready to emit
    pre_refs = {}    # sections whose scores were emitted during phase 1

    def emit_score_pair(qg, h, c0, c1, at_refs):
        # one [128,1024] psum pair: chunk c0 in the low half, c1 high; a
        # single wide exp covers both valid regions (the unwritten diagonal
        # gap of c1 is exp'd harmlessly — those at-columns are never read)
        q0 = 512 * qg
        rel0 = max(0, 128 * c0 - q0)
        rel1 = max(0, 128 * c1 - q0)
        psc = ps.tile([P, 1024], F32, tag="sc", bufs=2)
        nc.tensor.matmul(
            psc[:, rel0:512], KT[:, 128 * c0 : 128 * (c0 + 1)],
            QT[h][:, q0 + rel0 : q0 + 512], start=True, stop=True,
        )
        nc.tensor.matmul(
            psc[:, 512 + rel1 : 1024], KT[:, 128 * c1 : 128 * (c1 + 1)],
            QT[h][:, q0 + rel1 : q0 + 512], start=True, stop=True,
        )
        at = atp.tile([P, 1024], BF16)
        nc.scalar.activation(at[:, rel0:1024], psc[:, rel0:1024], AF.Exp)
        for side, c in ((0, c0), (1, c1)):
            if c >= 4 * qg:  # diagonal chunk: mask the partial block
                jj = c - 4 * qg
                o = 512 * side + 128 * jj
                nc.vector.tensor_mul(
                    at[:, o : o + 128], at[:, o : o + 128], maskt[:]
                )
        at_refs[c0] = (at, 0)
        at_refs[c1] = (at, 512)

    def emit_av_block(qg, h, jj, at_refs, state):
        j = 4 * qg + jj
        if jj not in state["pav"]:
            t = ps.tile([P, 512], F32, tag="av", bufs=4, name="pavt")
            state["pav"][jj] = state["pav"][jj ^ 1] = t
        pavt = state["pav"][jj]
        pavt_bf = pavt.bitcast(BF16)
        o = 256 * (jj % 2)
        for c in range(j + 1):
            at, off = at_refs[c]
            nc.tensor.matmul(
                pavt[:, o : o + 129],
                at[:, off + 128 * jj : off + 128 * (jj + 1)],
                VnO[:, VW * c : VW * c + 129],
                start=c == 0, stop=c == j,
                skip_group_check=True,
            )
        rec = recp.tile([P, 1], F32)
        nc.vector.reciprocal(rec[:], pavt[:, o + 128 : o + 129])
        aot = aotp.tile([P, P], BF16)
        nc.vector.tensor_scalar_mul(aot[:], pavt[:, o : o + 128], rec[:])
        # transpose target: free 8B-aligned columns of this pav bank
        tgt = pavt_bf[:, 260:388] if jj % 2 == 0 else pavt_bf[:, 772:900]
        state["items"].append((aot, j, tgt))

    otr = outT.rearrange("(dc p) s -> p dc s", p=P)  # [128, 16, 2048]
    obpend = {}  # parity-pair staging: one DMA per two consecutive series

    def flush_transposes(p):
        # Deferred PE transposes of normalized [q, dv] blocks -> AO [dv, q],
        # issued one slot later so the in-order PE never stalls on the DVE
        # normalize chain. Targets are unused columns of the pav tiles
        # themselves (bitcast bf16 view) — no extra psum bank.
        if p is None:
            return
        h, items = p
        for aot, j, tgt in items:
            nc.tensor.transpose(tgt, aot[:], ident[:])
            nc.vector.tensor_scalar_add(AO[h][:, 128 * j : 128 * (j + 1)], tgt, 0.0)

    def emit_po(Dc, qtr, single=False):
        D0 = 128 * Dc
        po = ps.tile([P, 512], F32, tag="av", bufs=4)
        for hc in range(NHQ):
            nc.tensor.matmul(
                po[:], woc[:, D * hc + D0 : D * hc + D0 + 128],
                AO[hc][:, 512 * qtr : 512 * (qtr + 1)],
                start=hc == 0, stop=hc == 3,
            )
        # consecutive (even, odd) Dc of the same quarter share one ob tile
        # and go out in a single DMA — halves the serialized HWDGE triggers
        if Dc % 2 == 0 and not single:
            ob = obp.tile([P, 1024], BF16)
            obpend[(Dc + 1, qtr)] = ob
            nc.vector.tensor_scalar_add(ob[:, 0:512], po[:], 0.0)
            return
        ob = obpend.pop((Dc, qtr), None)
        if ob is None:
            ob = obp.tile([P, 1024], BF16)
            nc.vector.tensor_scalar_add(ob[:, 0:512], po[:], 0.0)
            nc.sync.dma_start(
                otr[:, Dc : Dc + 1, 512 * qtr : 512 * (qtr + 1)], ob[:, 0:512]
            )
            return
        nc.vector.tensor_scalar_add(ob[:, 512:1024], po[:], 0.0)
        nc.sync.dma_start(
            otr[:, Dc - 1 : Dc + 1, 512 * qtr : 512 * (qtr + 1)], ob[:]
        )

    def section_pairs(qg):
        # diagonal chunks first: their exp feeds the Pool masks that the
        # NEXT slot's AV blocks consume — get them through ACT early
        nchunks = 4 * (qg + 1)
        order = list(range(4 * qg, nchunks)) + list(range(4 * qg))
        return [(order[2 * p], order[2 * p + 1]) for p in range(nchunks // 2)]

    def pre_score_thunks(qg, h):
        # per-pair thunks so phase 1 can sprinkle them between its passes
        # (the psc ring is exp-paced; a contiguous burst would stall the PE)
        at_refs = [None] * (4 * (qg + 1))
        pre_refs[(qg, h)] = at_refs
        return [
            (lambda c0=c0, c1=c1: emit_score_pair(qg, h, c0, c1, at_refs))
            for c0, c1 in section_pairs(qg)
        ]

    def run():
        pend = None      # transposes deferred from the AV one slot back
        prev = None      # (qg, h, at_refs) section awaiting AV
        for idx, (qg, h) in enumerate(secs):
            # quarter qg-1's AO completes with the transposes flushed in the
            # (qg, 1) slot; queue its output-projection series then
            if h == 1 and qg >= 1:
                po_queue.extend((Dc, qg - 1) for Dc in range(16))
            flush_transposes(pend)
            pend = None
            pre = pre_refs.get((qg, h))
            at_refs = pre if pre is not None else [None] * (4 * (qg + 1))
            pairs = [] if pre is not None else section_pairs(qg)
            if prev is None:
                for c0, c1 in pairs:
                    emit_score_pair(qg, h, c0, c1, at_refs)
            else:
                pqg, ph, pat = prev
                state = {"items": [], "pav": {}}
                # AV blocks lead each burst (their inputs are a slot old, so
                # the in-order PE never stalls); this section's score pairs
                # follow, paced behind them while ACT drains the exp backlog
                base, extra = divmod(len(pairs), 4)
                quota = [base + (1 if g < extra else 0) for g in range(4)]
                pi = 0
                for gi, g in enumerate((0, 2, 1, 3)):
                    emit_av_block(pqg, ph, g, pat, state)
                    # output-projection series fill the PE slack while ACT
                    # chews the exp backlog; qg3 slots are ACT-bound so they
                    # absorb an extra one
                    npo = (2 if qg >= 2 else 1) if gi % 2 == 1 else 0
                    for _ in range(npo):
                        if po_queue:
                            emit_po(*po_queue.pop(0))
                    for _ in range(quota[gi]):
                        emit_score_pair(qg, h, *pairs[pi], at_refs)
                        pi += 1
                pend = (ph, state["items"])
            prev = (qg, h, at_refs)

        # final section's AV + flush, then the remaining output projection
        flush_transposes(pend)
        pqg, ph, pat = prev
        state = {"items": [], "pav": {}}
        for g in (0, 2, 1, 3):
            emit_av_block(pqg, ph, g, pat, state)
            if po_queue:
                emit_po(*po_queue.pop(0))
        flush_transposes((ph, state["items"]))
        po_queue.extend((Dc, 3) for Dc in range(16))
        # last two series stay unpaired: a small final DMA starts sooner,
        # trimming the drain tail
        for Dc, qtr in po_queue[:-2]:
            emit_po(Dc, qtr)
        for Dc, qtr in po_queue[-2:]:
            emit_po(Dc, qtr, single=True)

    return pre_score_thunks, run


_NC_CACHE = {}


def _get_nc(reps=1):
    if reps in _NC_CACHE:
        return _NC_CACHE[reps]
    nc = bacc.Bacc("TRN2", target_bir_lowering=False, debug=False)
    aps = {}
    for name, shape, dt in [
        ("xT", [D, S], BF16), ("wq", [D, 512], BF16), ("wkv", [D, 2 * P], BF16),
        ("wo", [512, D], BF16), ("cos2", [P, S], BF16), ("sin2n", [P, S], BF16),
    ]:
        aps[name] = nc.dram_tensor(name, shape, dt, kind="ExternalInput").ap()
    outT = nc.dram_tensor("outT", [D, S], BF16, kind="ExternalOutput").ap()
    with tile.TileContext(nc) as tc, ExitStack() as ctx:
        if reps == 1:
            _build_kernel(
                nc, tc, ctx, aps["xT"], aps["wq"], aps["wkv"], aps["wo"],
                aps["cos2"], aps["sin2n"], outT,
            )
        else:
            with tc.For_i(0, reps, 1):
                with ExitStack() as inner:
                    _build_kernel(
                        nc, tc, inner, aps["xT"], aps["wq"], aps["wkv"],
                        aps["wo"], aps["cos2"], aps["sin2n"], outT,
                    )
    nc.compile()
    _NC_CACHE[reps] = nc
    return nc


def _prep_in_maps(x, freqs_cos, freqs_sin, w_q, w_k, w_v, w_o):
    bf = ml_dtypes.bfloat16
    x = np.asarray(x, np.float32)
    cosT = np.asarray(freqs_cos, np.float32).T  # [64, S]
    sinT = np.asarray(freqs_sin, np.float32).T
    cos2 = np.ascontiguousarray(np.concatenate([cosT, cosT], 0)).astype(bf)
    # negated-sin table: rows 0:64 = -sin (imag-half product), 64:128 = +sin
    sin2n = np.ascontiguousarray(np.concatenate([-sinT, sinT], 0)).astype(bf)
    w_q = np.asarray(w_q, np.float32)
    w_k = np.asarray(w_k, np.float32)
    w_v = np.asarray(w_v, np.float32)
    w_o = np.asarray(w_o, np.float32)

    # deinterleave head_dim: evens then odds (consistent for q and k)
    perm1 = np.concatenate([np.arange(0, P, 2), np.arange(1, P, 2)])
    in_maps = []
    for core in range(N_CORES):
        b, tp = divmod(core, N_TP)
        qcols = np.concatenate([4 * tp * P + h * P + perm1 for h in range(NHQ)])
        kcols = tp * P + perm1
        wq_c = np.ascontiguousarray(w_q[:, qcols] * (P ** -0.5)).astype(bf)
        wkv_c = np.ascontiguousarray(np.concatenate(
            [w_k[:, kcols], w_v[:, tp * P : (tp + 1) * P]], axis=1)).astype(bf)
        wo_c = np.ascontiguousarray(w_o[4 * tp * P : 4 * (tp + 1) * P, :]).astype(bf)
        xTc = np.ascontiguousarray(x[b].T).astype(bf)
        in_maps.append({
            "xT": xTc, "wq": wq_c, "wkv": wkv_c, "wo": wo_c,
            "cos2": cos2, "sin2n": sin2n,
        })
    return in_maps


def kernel(x, freqs_cos, freqs_sin, w_q, w_k, w_v, w_o):
    nc = _get_nc()
    in_maps = _prep_in_maps(x, freqs_cos, freqs_sin, w_q, w_k, w_v, w_o)
    results = run_bass_kernel_spmd(nc, in_maps, list(range(N_CORES))).results
    B = 2
    out = np.zeros((B, S, D), np.float32)
    for core in range(N_CORES):
        out[core // N_TP] += results[core]["outT"].astype(np.float32).T
    return out


# revision 60
# speedup vs baseline: 1.2446x; 1.2446x over previous
"""Trainium2 Bass kernel for GQA multi-head attention (B=2, S=2048, D=2048,
16 Q heads / 4 KV heads, head_dim=128, RoPE, causal). bf16 compute, fp32 accum.

Sharding: 8 cores = 2 (batch) x 4 (tensor-parallel head groups).
Each core: 4 Q heads + 1 KV head for one batch element; partial output
projection [D, S]; host sums the 4 TP partials per batch element.

Per-core dataflow (bf16 in SBUF, fp32 in PSUM; PSUM = one pool, two tags:
sc 2x[128,1024], av 4x[128,512] — rings shared across phases so nothing
ever waits on a pool boundary):

  opening: DMAs are consolidated into few large transfers (HWDGE costs
    ~630ns per dma_start, serialized) issued in consumption order on the
    in-order sync queue; sq0's x/wq stream in 4-chunk groups. ~10 junk
    matmuls on a zeroed tile warm the PE's HAM clock gate while the
    opening DMAs land.
  phase 1 (per s-quarter sq): sq0 runs chunk-major (all 4 Q products per
    dc-chunk as it arrives, accumulators in the sc pairs); sq1-3 run six
    sequential single-bank passes (Q0..Q3, K, then V). V is computed
    directly in natural [s, dv] blocks (stationary = xt block, moving =
    wv chunk; one psum alloc per block — group tracking is bank-granular)
    so no PE transposes are needed. Q/K drains alternate engines
    (ACT drain + Pool rope, or DVE drain + DVE rope):
      qs/qsw copies then dst = qs*cos + swap(qs)*sin_negated.
    K's drain is emitted after the V pass so VnO copies hit an idle ACT.
    During sq3, the qg0 sections' score pairs are sprinkled between
    passes so their exps retire early and phase 2 opens with AV work.
  phase 2 (per section (qg, h), software-pipelined slots): slot t emits
    AV blocks of sec_{t-1} (order 0,2,1,3 so consecutive blocks hit
    different psum banks), out-projection series, and scores(sec_t),
    keeping the PE busy while ACT works through the exp backlog.
    scores: per 128-k-chunk-PAIR [128, 1024] psum, diagonal chunks first;
    one wide exp covers both valid regions (per-instr ACT overhead is
    ~185ns, pairing halves it); diagonal blocks masked on Pool.
    AV flipped: pav[q, 129] += at_block.T @ VnO_chunk[dv | ones]
      -> softmax denominator rides in column 128 for free.
    normalize: aot = pav[:,0:128] * reciprocal(pav[:,128]) (DVE);
    deferred PE-transpose aot -> AO[h] [dv, q] one slot later (targets are
    spare columns of the pav banks, bitcast bf16).
  phase 3: outT[Dc, 512q] = sum_hc wo_chunk.T @ AO[hc], emitted as soon as
    each q-quarter's AO completes, interleaved between phase-2 AV blocks;
    drains on DVE; consecutive Dc pairs share one output DMA.

Output: per-core outT [D, S] bf16; host sums out[b] = sum_tp outT.T
"""

import numpy as np
from contextlib import ExitStack

import ml_dtypes
import concourse.bass as bass
import concourse.mybir as mybir
from concourse import bacc, tile
from concourse.bass_utils import run_bass_kernel_spmd
from concourse.masks import make_identity, make_upper_triangular

F32 = mybir.dt.float32
BF16 = mybir.dt.bfloat16
AF = mybir.ActivationFunctionType

S = 2048
D = 2048
P = 128
NHQ = 4   # q heads per core
N_CORES = 8
N_TP = 4
VW = 132  # VnO column stride per k-chunk: 128 dv + 1 ones + 3 pad


def _build_kernel(nc, tc, ctx, xT, wq, wkv, wo, cos2, sin2n, outT):
    const = ctx.enter_context(tc.tile_pool(name="const", bufs=1))
    xtp = ctx.enter_context(tc.tile_pool(name="xtp", bufs=2))
    sbp = ctx.enter_context(tc.tile_pool(name="sbp", bufs=8))
    ropep = ctx.enter_context(tc.tile_pool(name="ropep", bufs=6))
    atp = ctx.enter_context(tc.tile_pool(name="atp", bufs=16))
    aotp = ctx.enter_context(tc.tile_pool(name="aotp", bufs=9))
    recp = ctx.enter_context(tc.tile_pool(name="recp", bufs=9))
    obp = ctx.enter_context(tc.tile_pool(name="obp", bufs=6))

    # ---- persistent weights / tables ----
    wqc = const.tile([P, 16 * 512], BF16, tag="wqc")
    wkvc = const.tile([P, 16 * 256], BF16, tag="wkvc")
    cos2t = const.tile([P, S], BF16, tag="cos2t")
    sin2nt = const.tile([P, S], BF16, tag="sin2nt")
    woc = const.tile([P, 4 * D], BF16, tag="woc")

    # DMA kickoff in consumption order, consolidated into few transfers (the
    # HWDGE descriptor engine costs ~630ns per dma_start, serialized): sq0's
    # x and wq stream in 4-chunk groups so pass A starts early; everything
    # else is one transfer. HBM-side chunking via einops rearrange.
    wup0 = const.tile([P, 512], BF16, tag="wup")
    nc.gpsimd.memset(wup0[:], 0.0)

    wqr = wq.rearrange("(dc p) c -> p dc c", p=P)      # [128, 16, 512]
    wkvr = wkv.rearrange("(dc p) c -> p dc c", p=P)    # [128, 16, 256]
    wor = wo.rearrange("(hc p) c -> p hc c", p=P)      # [128, 4, 2048]
    xtr = xT.rearrange("(dc p) s -> p dc s", p=P)      # [128, 16, 2048]
    xtb = {0: xtp.tile([P, 16 * 512], BF16, tag="xtb", name="xtb0")}
    for g in range(4):
        nc.sync.dma_start(
            xtb[0][:, 2048 * g : 2048 * (g + 1)],
            xtr[:, 4 * g : 4 * (g + 1), 0:512],
        )
        nc.sync.dma_start(
            wqc[:, 2048 * g : 2048 * (g + 1)], wqr[:, 4 * g : 4 * (g + 1), :]
        )
    nc.sync.dma_start(wkvc[:], wkvr[:])
    nc.sync.dma_start(cos2t[:], cos2[:])
    nc.sync.dma_start(sin2nt[:], sin2n[:])

    # ---- PSUM: one pool, two tags; 8 banks total ----
    # sc: 2x[128,1024] (sq0 Q pairs, then score chunk-pairs)
    # av: 4x[128,512] (phase-1 accumulators/V blocks, then AV + out-proj)
    ps = ctx.enter_context(tc.tile_pool(name="ps", bufs=1, space="PSUM"))

    # PE warm-up: the tensor engine's HAM clock gate holds the PE at half
    # clock until ~3.4us of sustained activity. The opening is DMA-paced, so
    # burn that window on junk matmuls (zeros) instead of on the real ones.
    wup = wup0
    pw = ps.tile([P, 512], F32, tag="av", bufs=4, name="pw")
    for i in range(8):12        nc.tensor.matmul(12            pw[:], wup[:, 0:128], wup[:], start=True, stop=True,
            skip_group_check=True,
        )

    maskt = const.tile([P, P], BF16, tag="maskt")
    make_upper_triangular(nc, maskt[:], val=1.0, diag=True)
    ident = const.tile([P, P], BF16, tag="ident")
    make_identity(nc, ident[:])

    QT = [const.tile([P, S], BF16, tag=f"QT{i}", name=f"QT{i}") for i in range(NHQ)]
    KT = const.tile([P, S], BF16, tag="KT")
    # VnO: per k-chunk c: cols [VW*c, VW*c+128) = V rows [128c,+128) x dv,
    # col VW*c+128 = ones (softmax denominator rides the AV matmul)
    VnO = const.tile([P, 16 * VW], BF16, tag="VnO")
    for c in range(16):
        nc.gpsimd.memset(VnO[:, VW * c + 128 : VW * c + 129], 1.0)
    AO = [const.tile([P, S], BF16, tag=f"AO{i}", name=f"AO{i}") for i in range(NHQ)]

    pre_score_thunks, run_phase23 = _make_phase23(
        nc, ps, atp, aotp, recp, obp, maskt, ident, QT, KT, VnO, AO, woc, outT
    )
    late_work = []
    _phase1(nc, ps, xtp, sbp, ropep, xtr, xtb, wqc, wkvc, cos2t, sin2nt,
            QT, KT, VnO, wor, woc, pre_score_thunks, late_work)
    run_phase23(late_work)


def _phase1(nc, ps, xtp, sbp, ropep, xtr, xtb, wqc, wkvc, cos2t, sin2nt,
            QT, KT, VnO, wor, woc, pre_score_thunks, late_work):
    def drain_rope(psrc, dst, sl, alt):
        # psum -> bf16 sbuf: natural copy + half-swapped copy, then RoPE as
        # 3 full-width all-bf16 ops: dst = qs*cos + swap(qs)*sin_negated.
        # Engines alternate per tensor (drain ACT + rope Pool, or both DVE)
        # so no single engine's queue backlog stalls the psum bank recycle.
        qs = sbp.tile([P, 512], BF16, tag="qs")
        qsw = sbp.tile([P, 512], BF16, tag="qsw")
        if alt % 2 == 0:
            nc.scalar.activation(qs[:], psrc, AF.Copy)
            nc.scalar.activation(qsw[0:64, :], psrc[64:128, :], AF.Copy)
            nc.scalar.activation(qsw[64:128, :], psrc[0:64, :], AF.Copy)
            reng = nc.gpsimd
        else:
            nc.vector.tensor_scalar_add(qs[:], psrc, 0.0)
            nc.vector.tensor_scalar_add(qsw[0:64, :], psrc[64:128, :], 0.0)
            nc.vector.tensor_scalar_add(qsw[64:128, :], psrc[0:64, :], 0.0)
            reng = nc.vector
        m1 = ropep.tile([P, 512], BF16, tag="m1")
        m2 = ropep.tile([P, 512], BF16, tag="m2")
        reng.tensor_mul(m1[:], qs[:], cos2t[:, sl])
        reng.tensor_mul(m2[:], qsw[:], sin2nt[:, sl])
        reng.tensor_add(dst[:, sl], m1[:], m2[:])

    for sq in range(4):
        if sq == 3:
            # qg0 sections need only sq0 data: their score pairs get
            # sprinkled between sq3's passes below, so the exps retire on
            # ACT during sq3 and phase 2 opens with ready AV work
            pre = []
            for h0 in range(NHQ):
                pre.extend(pre_score_thunks(0, h0))
        else:
            pre = []
        s0 = 512 * sq
        sl = slice(s0, s0 + 512)
        # prefetch next sq's x square in one transfer (xtb double-buffers)
        if sq < 3:
            xtb[sq + 1] = xtp.tile([P, 16 * 512], BF16, tag="xtb",
                                   name=f"xtb{sq+1}")
            nc.sync.dma_start(
                xtb[sq + 1][:], xtr[:, :, 512 * (sq + 1) : 512 * (sq + 2)]
            )
        if sq == 1:
            # stream the phase-3 weights while the DMA queues are quiet
            # (on sync: its in-order queue keeps this behind the opening)
            nc.sync.dma_start(woc[:], wor[:])
        xtq = xtb.pop(sq)
        xtl = [xtq[:, 512 * dc : 512 * (dc + 1)] for dc in range(16)]

        def pass_V():
            # pass V: natural [s, dv] blocks; stationary = xt block, moving
            # = wv. Each t-block gets its own psum alloc: accumulation-group
            # tracking is bank-granular, so sharing one tile would serialize
            # block t+1's matmuls behind block t's VnO drain.
            for t in range(4):
                pVt = ps.tile([P, 512], F32, tag="av", bufs=4, name="pVt")
                for dc in range(16):
                    nc.tensor.matmul(
                        pVt[:, 0:128],
                        xtq[:, 512 * dc + 128 * t : 512 * dc + 128 * (t + 1)],
                        wkvc[:, 256 * dc + 128 : 256 * dc + 256],
                        start=dc == 0, stop=dc == 15, skip_group_check=True,
                    )
                j = 4 * sq + t
                if t % 2 == 0:
                    nc.scalar.activation(
                        VnO[:, VW * j : VW * j + 128], pVt[:, 0:128], AF.Copy,
                    )
                else:
                    nc.vector.tensor_scalar_add(
                        VnO[:, VW * j : VW * j + 128], pVt[:, 0:128], 0.0,
                    )

        if sq == 0:
            # sq0 is paced by the opening DMA stream: consume chunks in
            # chunk-major order (all 4 Q products per chunk as it arrives).
            # Borrows the phase-2 'sc' tag for the 4 live accumulators —
            # phase 2 is far away and the drains finish during sq1.
            pQp = [
                ps.tile([P, 1024], F32, tag="sc", bufs=2, name=f"pQp{i}")
                for i in range(2)
            ]
            qt = [pQp[h // 2][:, 512 * (h % 2) : 512 * (h % 2) + 512]
                  for h in range(NHQ)]
            for dc in range(16):
                for h in range(NHQ):
                    nc.tensor.matmul(
                        qt[h],
                        wqc[:, 512 * dc + 128 * h : 512 * dc + 128 * (h + 1)],
                        xtl[dc], start=dc == 0, stop=dc == 15,
                        skip_group_check=True,
                    )
            for h in range(NHQ):
                drain_rope(qt[h], QT[h], sl, h)
        else:
            # passes Q0..Q3: single-bank accumulation, drain+rope per pass
            for h in range(NHQ):
                pQ = ps.tile([P, 512], F32, tag="av", bufs=4, name=f"pQ{h}")
                for dc in range(16):
                    nc.tensor.matmul(
                        pQ[:],
                        wqc[:, 512 * dc + 128 * h : 512 * dc + 128 * (h + 1)],
                        xtl[dc], start=dc == 0, stop=dc == 15,
                    )
                drain_rope(pQ[:], QT[h], sl, h)
                for _ in range(2):
                    if pre:
                        pre.pop(0)()
        def k_pass(drain_alt=0, vno_eng=None):
            pK = ps.tile([P, 512], F32, tag="av", bufs=4, name="pK")
            for dc in range(16):
                nc.tensor.matmul(
                    pK[:], wkvc[:, 256 * dc : 256 * dc + 128], xtl[dc],
                    start=dc == 0, stop=dc == 15,
                )
            return pK

        if sq < 3:
            pK = k_pass()
            for _ in range(2):
                if pre:
                    pre.pop(0)()
            pass_V()
            # K's drain is emitted after the V pass so the VnO copies reach
            # an empty ACT queue (KT isn't needed until phase 2)
            drain_rope(pK[:], KT, sl, 0)
            for t in pre:
                t()
        else:
            # sq3's K/V products feed only the late phase-2 sections: defer
            # them into the early (nearly empty) phase-2 slots, pulling the
            # attention pipeline ~7us earlier.  K's drain precedes the V
            # blocks so the 'av' ring never waits on a later-emitted read;
            # VnO copies go to DVE (ACT is busy with exps there).
            st = {}

            def _lk(g):
                # quarter of the K pass: 4 matmuls — fine-grained fillers
                # for the latency-bound early phase-2 AV slots
                if g == 0:
                    st["pK"] = ps.tile([P, 512], F32, tag="av", bufs=4,
                                       name="pK")
                for dc in range(4 * g, 4 * g + 4):
                    nc.tensor.matmul(
                        st["pK"][:], wkvc[:, 256 * dc : 256 * dc + 128],
                        xtl[dc], start=dc == 0, stop=dc == 15,
                    )

            def _lkd():
                drain_rope(st["pK"][:], KT, sl, 1)

            def _lv(t):
                pVt = ps.tile([P, 512], F32, tag="av", bufs=4, name="pVt")
                for dc in range(16):
                    nc.tensor.matmul(
                        pVt[:, 0:128],
                        xtq[:, 512 * dc + 128 * t : 512 * dc + 128 * (t + 1)],
                        wkvc[:, 256 * dc + 128 : 256 * dc + 256],
                        start=dc == 0, stop=dc == 15, skip_group_check=True,
                    )
                j = 12 + t
                nc.vector.tensor_scalar_add(
                    VnO[:, VW * j : VW * j + 128], pVt[:, 0:128], 0.0
                )

            late_work.extend(
                [(lambda g=g: _lk(g)) for g in range(4)]
                + [_lkd] + [(lambda t=t: _lv(t)) for t in range(4)]
            )
            for t in pre:
                t()


def _make_phase23(nc, ps, atp, aotp, recp, obp, maskt, ident, QT, KT, VnO,
                  AO, woc, outT):
    secs = [(qg, h) for qg in range(4) for h in range(4)]
    po_queue = []    # (Dc, qtr) output-projection series ready to emit
    pre_refs = {}    # sections whose scores were emitted during phase 1

    def emit_score_pair(qg, h, c0, c1, at_refs):
        # one [128,1024] psum pair: chunk c0 in the low half, c1 high; a
        # single wide exp covers both valid regions (the unwritten diagonal
        # gap of c1 is exp'd harmlessly — those at-columns are never read)
        q0 = 512 * qg
        rel0 = max(0, 128 * c0 - q0)
        rel1 = max(0, 128 * c1 - q0)
        psc = ps.tile([P, 1024], F32, tag="sc", bufs=2)
        nc.tensor.matmul(
            psc[:, rel0:512], KT[:, 128 * c0 : 128 * (c0 + 1)],
            QT[h][:, q0 + rel0 : q0 + 512], start=True, stop=True,
        )
        nc.tensor.matmul(
            psc[:, 512 + rel1 : 1024], KT[:, 128 * c1 : 128 * (c1 + 1)],
            QT[h][:, q0 + rel1 : q0 + 512], start=True, stop=True,
        )
        at = atp.tile([P, 1024], BF16)
        nc.scalar.activation(at[:, rel0:1024], psc[:, rel0:1024], AF.Exp)
        for side, c in ((0, c0), (1, c1)):
            if c >= 4 * qg:  # diagonal chunk: mask the partial block
                jj = c - 4 * qg
                o = 512 * side + 128 * jj
                nc.vector.tensor_mul(
                    at[:, o : o + 128], at[:, o : o + 128], maskt[:]
                )
        at_refs[c0] = (at, 0)
        at_refs[c1] = (at, 512)

    def emit_av_block(qg, h, jj, at_refs, state):
        j = 4 * qg + jj
        if jj not in state["pav"]:
            t = ps.tile([P, 512], F32, tag="av", bufs=4, name="pavt")
            state["pav"][jj] = state["pav"][jj ^ 1] = t
        pavt = state["pav"][jj]
        pavt_bf = pavt.bitcast(BF16)
        o = 256 * (jj % 2)
        for c in range(j + 1):
            at, off = at_refs[c]
            nc.tensor.matmul(
                pavt[:, o : o + 129],
                at[:, off + 128 * jj : off + 128 * (jj + 1)],
                VnO[:, VW * c : VW * c + 129],
                start=c == 0, stop=c == j,
                skip_group_check=True,
            )
        rec = recp.tile([P, 1], F32)
        nc.vector.reciprocal(rec[:], pavt[:, o + 128 : o + 129])
        aot = aotp.tile([P, P], BF16)
        nc.vector.tensor_scalar_mul(aot[:], pavt[:, o : o + 128], rec[:])
        # transpose target: free 8B-aligned columns of this pav bank
        tgt = pavt_bf[:, 260:388] if jj % 2 == 0 else pavt_bf[:, 772:900]
        state["items"].append((aot, j, tgt))

    otr = outT.rearrange("(dc p) s -> p dc s", p=P)  # [128, 16, 2048]
    obpend = {}  # parity-pair staging: one DMA per two consecutive series

    def flush_transposes(p):
        # Deferred PE transposes of normalized [q, dv] blocks -> AO [dv, q],
        # issued one slot later so the in-order PE never stalls on the DVE
        # normalize chain. Targets are unused columns of the pav tiles
        # themselves (bitcast bf16 view) — no extra psum bank.
        if p is None:
            return
        h, items = p
        for aot, j, tgt in items:
            nc.tensor.transpose(tgt, aot[:], ident[:])
            nc.vector.tensor_scalar_add(AO[h][:, 128 * j : 128 * (j + 1)], tgt, 0.0)

    def emit_po(Dc, qtr, single=False, ob_act=False):
        D0 = 128 * Dc
        po = ps.tile([P, 512], F32, tag="av", bufs=4)
        for hc in range(NHQ):
            nc.tensor.matmul(
                po[:], woc[:, D * hc + D0 : D * hc + D0 + 128],
                AO[hc][:, 512 * qtr : 512 * (qtr + 1)],
                start=hc == 0, stop=hc == 3,
            )

        def drain(dst, src):
            if ob_act:
                nc.scalar.activation(dst, src, AF.Copy)
            else:
                nc.vector.tensor_scalar_add(dst, src, 0.0)

        # consecutive (even, odd) Dc of the same quarter share one ob tile
        # and go out in a single DMA — halves the serialized HWDGE triggers
        if Dc % 2 == 0 and not single:
            ob = obp.tile([P, 1024], BF16)
            obpend[(Dc + 1, qtr)] = ob
            drain(ob[:, 0:512], po[:])
            return
        ob = obpend.pop((Dc, qtr), None)
        if ob is None:
            ob = obp.tile([P, 1024], BF16)
            drain(ob[:, 0:512], po[:])
            nc.sync.dma_start(
                otr[:, Dc : Dc + 1, 512 * qtr : 512 * (qtr + 1)], ob[:, 0:512]
            )
            return
        drain(ob[:, 512:1024], po[:])
        nc.sync.dma_start(
            otr[:, Dc - 1 : Dc + 1, 512 * qtr : 512 * (qtr + 1)], ob[:]
        )

    def section_pairs(qg):
        # diagonal chunks first: their exp feeds the Pool masks that the
        # NEXT slot's AV blocks consume — get them through ACT early
        nchunks = 4 * (qg + 1)
        order = list(range(4 * qg, nchunks)) + list(range(4 * qg))
        return [(order[2 * p], order[2 * p + 1]) for p in range(nchunks // 2)]

    def pre_score_thunks(qg, h):
        # per-pair thunks so phase 1 can sprinkle them between its passes
        # (the psc ring is exp-paced; a contiguous burst would stall the PE)
        at_refs = [None] * (4 * (qg + 1))
        pre_refs[(qg, h)] = at_refs
        return [
            (lambda c0=c0, c1=c1: emit_score_pair(qg, h, c0, c1, at_refs))
            for c0, c1 in section_pairs(qg)
        ]

    def run(late_work):
        pend = None      # transposes deferred from the AV one slot back
        prev = None      # (qg, h, at_refs) section awaiting AV
        late_sched = {1: 4, 2: 2, 3: 2, 4: 1}  # slot idx -> thunks to emit
        for idx, (qg, h) in enumerate(secs):
            # quarter qg-1's AO completes with the transposes flushed in the
            # (qg, 1) slot; queue its output-projection series then
            if h == 1 and qg >= 1:
                po_queue.extend((Dc, qg - 1) for Dc in range(16))
            flush_transposes(pend)
            pend = None
            pre = pre_refs.get((qg, h))
            at_refs = pre if pre is not None else [None] * (4 * (qg + 1))
            pairs = [] if pre is not None else section_pairs(qg)
            if prev is None:
                for c0, c1 in pairs:
                    emit_score_pair(qg, h, c0, c1, at_refs)
            else:
                pqg, ph, pat = prev
                state = {"items": [], "pav": {}}
                # AV blocks lead each burst (their inputs are a slot old, so
                # the in-order PE never stalls); this section's score pairs
                # follow, paced behind them while ACT drains the exp backlog
                base, extra = divmod(len(pairs), 4)
                quota = [base + (1 if g < extra else 0) for g in range(4)]
                pi = 0
                for gi, g in enumerate((0, 2, 1, 3)):
                    emit_av_block(pqg, ph, g, pat, state)
                    # output-projection series fill the PE slack while ACT
                    # chews the exp backlog; qg3 slots are ACT-bound so they
                    # absorb an extra one
                    npo = (2 if qg >= 2 else 1) if gi % 2 == 1 else 0
                    for _ in range(npo):
                        if po_queue:
                            emit_po(*po_queue.pop(0))
                    for _ in range(quota[gi]):
                        emit_score_pair(qg, h, *pairs[pi], at_refs)
                        pi += 1
                pend = (ph, state["items"])
            prev = (qg, h, at_refs)
            for _ in range(late_sched.get(idx, 0)):
                if late_work:
                    late_work.pop(0)()

        # final section's AV + flush, then the remaining output projection
        flush_transposes(pend)
        pqg, ph, pat = prev
        state = {"items": [], "pav": {}}
        for g in (0, 2, 1, 3):
            emit_av_block(pqg, ph, g, pat, state)
            if po_queue:
                emit_po(*po_queue.pop(0))
        flush_transposes((ph, state["items"]))
        po_queue.extend((Dc, 3) for Dc in range(16))
        # last two series stay unpaired: a small final DMA starts sooner,
        # trimming the drain tail; tail obs alternate ACT/DVE (ACT is idle
        # once the exps are done)
        for Dc, qtr in po_queue[:-2]:
            emit_po(Dc, qtr)
        for i, (Dc, qtr) in enumerate(po_queue[-2:]):
            emit_po(Dc, qtr, single=True, ob_act=(i == 0))

    return pre_score_thunks, run


_NC_CACHE = {}


def _get_nc(reps=1):
    if reps in _NC_CACHE:
        return _NC_CACHE[reps]
    nc = bacc.Bacc("TRN2", target_bir_lowering=False, debug=False)
    aps = {}
    for name, shape, dt in [
        ("xT", [D, S], BF16), ("wq", [D, 512], BF16), ("wkv", [D, 2 * P], BF16),
        ("wo", [512, D], BF16), ("cos2", [P, S], BF16), ("sin2n", [P, S], BF16),
    ]:
        aps[name] = nc.dram_tensor(name, shape, dt, kind="ExternalInput").ap()
    outT = nc.dram_tensor("outT", [D, S], BF16, kind="ExternalOutput").ap()
    with tile.TileContext(nc) as tc, ExitStack() as ctx:
        if reps == 1:
            _build_kernel(
                nc, tc, ctx, aps["xT"], aps["wq"], aps["wkv"], aps["wo"],
                aps["cos2"], aps["sin2n"], outT,
            )
        else:
            with tc.For_i(0, reps, 1):
                with ExitStack() as inner:
                    _build_kernel(
                        nc, tc, inner, aps["xT"], aps["wq"], aps["wkv"],
                        aps["wo"], aps["cos2"], aps["sin2n"], outT,
                    )
    nc.compile()
    _NC_CACHE[reps] = nc
    return nc


def _prep_in_maps(x, freqs_cos, freqs_sin, w_q, w_k, w_v, w_o):
    bf = ml_dtypes.bfloat16
    x = np.asarray(x, np.float32)
    cosT = np.asarray(freqs_cos, np.float32).T  # [64, S]
    sinT = np.asarray(freqs_sin, np.float32).T
    cos2 = np.ascontiguousarray(np.concatenate([cosT, cosT], 0)).astype(bf)
    # negated-sin table: rows 0:64 = -sin (imag-half product), 64:128 = +sin
    sin2n = np.ascontiguousarray(np.concatenate([-sinT, sinT], 0)).astype(bf)
    w_q = np.asarray(w_q, np.float32)
    w_k = np.asarray(w_k, np.float32)
    w_v = np.asarray(w_v, np.float32)
    w_o = np.asarray(w_o, np.float32)

    # deinterleave head_dim: evens then odds (consistent for q and k)
    perm1 = np.concatenate([np.arange(0, P, 2), np.arange(1, P, 2)])
    in_maps = []
    for core in range(N_CORES):
        b, tp = divmod(core, N_TP)
        qcols = np.concatenate([4 * tp * P + h * P + perm1 for h in range(NHQ)])
        kcols = tp * P + perm1
        wq_c = np.ascontiguousarray(w_q[:, qcols] * (P ** -0.5)).astype(bf)
        wkv_c = np.ascontiguousarray(np.concatenate(
            [w_k[:, kcols], w_v[:, tp * P : (tp + 1) * P]], axis=1)).astype(bf)
        wo_c = np.ascontiguousarray(w_o[4 * tp * P : 4 * (tp + 1) * P, :]).astype(bf)
        xTc = np.ascontiguousarray(x[b].T).astype(bf)
        in_maps.append({
            "xT": xTc, "wq": wq_c, "wkv": wkv_c, "wo": wo_c,
            "cos2": cos2, "sin2n": sin2n,
        })
    return in_maps


def kernel(x, freqs_cos, freqs_sin, w_q, w_k, w_v, w_o):
    nc = _get_nc()
    in_maps = _prep_in_maps(x, freqs_cos, freqs_sin, w_q, w_k, w_v, w_o)
    results = run_bass_kernel_spmd(nc, in_maps, list(range(N_CORES))).results
    B = 2
    out = np.zeros((B, S, D), np.float32)
    for core in range(N_CORES):
        out[core // N_TP] += results[core]["outT"].astype(np.float32).T
    return out
